# revision 2
# baseline (speedup 1.0000x reference)
"""LIF spiking-neuron kernel for Trainium2, data-parallel over 8 NeuronCores.

Reference semantics (T=4, THRESH=1.0, TAU=1.0):
    x: [T*B, N] -> reshape [T, B, N]; mem0 = 0
    per t: mem += x_t; spike_t = (mem >= 1.0); mem *= (1 - spike_t)
    out: spikes reshaped [T*B, N]

Sharding: pure data parallel over B. Core i gets rows i*256:(i+1)*256 of
each timestep block -> shard [T*256, N] = [1024, 4096] f32 in per core.

The kernel is DMA-bound, so spikes (exactly 0.0/1.0) are produced and
stored as fp8-e4m3 (lossless for {0,1}), cutting store traffic 4x vs f32;
the host upcasts to f32. Measured per-core DMA floor for this pattern
(16.78 MB f32 reads + 4.19 MB fp8 writes) is ~39 us vs ~91 us for the
f32-store pattern the old baseline used.

Engine split (raw Bass):
  SP (sync, HWDGE)   : x loads, ring-buffered
  ACT (scalar, HWDGE): fp8 spike stores (separate DGE ring)
  DVE (vector)       : per t: add (skipped at t=0: mem0+x0 = x0),
                       spike = is_ge(mem, 1) -> fp8,
                       reset fused in ONE op: mem = (mem < 1) * mem
                       (scalar_tensor_tensor; skipped at t=3, mem dead)
"""

from contextlib import ExitStack

import numpy as np

import concourse.bass as bass
from concourse import mybir
from concourse.bass_utils import run_bass_kernel_spmd

T = 4
B = 2048
N = 4096
N_CORES = 8
BSH = B // N_CORES  # 256 rows per core per timestep
P = 128

F32 = mybir.dt.float32
FP8 = mybir.dt.float8e4  # e4m3; 1.0 -> 0x38, 0.0 -> 0x00 (exact)


def build_nc(t_dim=T, bsh=BSH, n=N, bench_iters=None):
    """One-core Bass module: x [t*bsh, n] f32 -> out [t*bsh, n] fp8.

    bench_iters: if set, repeat the whole (idempotent) program that many
    times with continuing semaphore counts — used only for slope timing.
    """
    pb = bsh // P  # spatial chunks of [128, n]
    assert bsh % P == 0
    reps = bench_iters or 1
    ng = pb * reps  # chunk instances
    NXB = 6  # x-tile ring
    NSB = 4  # spike-tile ring

    nc = bass.Bass()
    x = nc.declare_dram_parameter("x", [t_dim * bsh, n], F32, isOutput=False)
    out = nc.declare_dram_parameter("out", [t_dim * bsh, n], FP8, isOutput=True)
    xv = x.rearrange("(t pb p) n -> t pb p n", t=t_dim, pb=pb, p=P)
    ov = out.rearrange("(t pb p) n -> t pb p n", t=t_dim, pb=pb, p=P)

    # --- precompute DVE program order so waits can reference exact counts.
    # v counts DVE instructions (each increments v_sem by 1).
    # Per chunk instance g: t0: ge, stt | t1: add, ge, stt | t2: add, ge,
    # stt | t3: add, ge   -> 10 ops.
    vidx_xfree = {}  # x-load j -> v count after the op that last reads it
    vidx_ge = {}  # spike unit u -> v count after its is_ge
    v = 0
    for g in range(ng):
        for t in range(t_dim):
            u = t_dim * g + t
            j = t_dim * g + t
            if t == 0:
                v += 2  # ge, stt (both read xb j directly)
                vidx_xfree[j] = v
                vidx_ge[u] = v - 1
            else:
                v += 1  # add (consumes xb j)
                vidx_xfree[j] = v
                v += 1  # ge
                vidx_ge[u] = v
                if t < t_dim - 1:
                    v += 1  # stt reset

    with ExitStack() as ctx:
        mem = [
            ctx.enter_context(nc.sbuf_tensor(f"mem{i}", [P, n], F32))
            for i in range(2)
        ]
        xb = [
            ctx.enter_context(nc.sbuf_tensor(f"xb{i}", [P, n], F32))
            for i in range(NXB)
        ]
        sb = [
            ctx.enter_context(nc.sbuf_tensor(f"sb{i}", [P, n], FP8))
            for i in range(NSB)
        ]
        # One semaphore per ring slot: concurrent DMA completions interleave
        # their 16 per-engine increments, so a shared cumulative sem cannot
        # identify which DMA finished.
        xb_sem = [
            ctx.enter_context(nc.semaphore(f"xb_sem{i}")) for i in range(NXB)
        ]
        sb_sem = [
            ctx.enter_context(nc.semaphore(f"sb_sem{i}")) for i in range(NSB)
        ]
        v_sem = ctx.enter_context(nc.semaphore("v_sem"))
        block = ctx.enter_context(nc.Block())

        @block.sync
        def _(sync):
            for j in range(t_dim * ng):
                g, t = divmod(j, t_dim)
                c = g % pb
                if j >= NXB:  # WAR: x slot still read by DVE op
                    sync.wait_ge(v_sem, vidx_xfree[j - NXB])
                sync.dma_start(xb[j % NXB][:], xv[t, c]).then_inc(
                    xb_sem[j % NXB], 16
                )

        @block.vector
        def _(vector):
            v = 0

            def dve(ins):
                nonlocal v
                v += 1
                ins.then_inc(v_sem, 1)

            for g in range(ng):
                m = mem[g % 2]
                for t in range(t_dim):
                    u = t_dim * g + t
                    j = t_dim * g + t
                    vector.wait_ge(xb_sem[j % NXB], 16 * (j // NXB + 1))
                    src = xb[j % NXB] if t == 0 else m
                    if t > 0:
                        vector.wait_ge(v_sem, v)
                        dve(vector.tensor_add(m[:], m[:], xb[j % NXB][:]))
                    if u >= NSB:  # WAR: spike slot still being stored
                        vector.wait_ge(sb_sem[u % NSB], 16 * (u // NSB))
                    vector.wait_ge(v_sem, v)
                    dve(
                        vector.tensor_scalar(
                            sb[u % NSB][:], src[:], 1.0, None,
                            mybir.AluOpType.is_ge,
                        )
                    )
                    if t < t_dim - 1:
                        vector.wait_ge(v_sem, v)
                        dve(
                            vector.scalar_tensor_tensor(
                                m[:], src[:], 1.0, src[:],
                                mybir.AluOpType.is_lt, mybir.AluOpType.mult,
                            )
                        )

        @block.scalar
        def _(scalar):
            nu = t_dim * ng
            for u in range(nu):
                g, t = divmod(u, t_dim)
                c = g % pb
                scalar.wait_ge(v_sem, vidx_ge[u])
                scalar.dma_start(ov[t, c], sb[u % NSB][:]).then_inc(
                    sb_sem[u % NSB], 16
                )
            for i in range(NSB):  # drain: all stores landed before NEFF end
                scalar.wait_ge(sb_sem[i], 16 * ((nu - 1 - i) // NSB + 1))

    return nc


_NC_CACHE = None


def _get_nc():
    global _NC_CACHE
    if _NC_CACHE is None:
        _NC_CACHE = build_nc()
    return _NC_CACHE


def shard_input(x):
    """x [T*B, N] -> list of 8 shards [T*BSH, N], C-contiguous."""
    xs = x.reshape(T, B, N)
    return [
        np.ascontiguousarray(xs[:, i * BSH : (i + 1) * BSH, :]).reshape(T * BSH, N)
        for i in range(N_CORES)
    ]


def unshard_output(results):
    """8 fp8/uint8-viewable shards [T*BSH, N] -> full f32 [T*B, N].

    Spikes were stored as fp8-e4m3: 1.0 -> 0x38, 0.0 -> 0x00.
    """
    out = np.empty((T, B, N), dtype=np.float32)
    for i in range(N_CORES):
        raw = np.asarray(results[i]).view(np.uint8).reshape(T, BSH, N)
        out[:, i * BSH : (i + 1) * BSH, :] = (raw == 0x38).astype(np.float32)
    return out.reshape(T * B, N)


def run_sharded(x, trace=False):
    nc = _get_nc()
    in_maps = [{"x": s} for s in shard_input(x)]
    res = run_bass_kernel_spmd(nc, in_maps, list(range(N_CORES)), trace=trace)
    return unshard_output([r["out"] for r in res.results]), res


def kernel(x):
    x = np.asarray(x, dtype=np.float32)
    assert x.shape == (T * B, N)
    out, _ = run_sharded(x, trace=False)
    return out
